# revision 13
# baseline (speedup 1.0000x reference)
"""AutoCorrelation kernel for Trainium2 (Bass/Tile), 8-core data parallel.

Math: the reference computes rfft over the zero-padded head dim (D=64 -> L=512),
multiplies conj(Q)*K, irffts, then MEANS over heads AND the whole lag axis.
Summing a circular correlation over all lags factorizes:
    sum_t corr[t] = (sum_d q[d]) * (sum_d k[d])
so  x_corr_mean[b,l] = 1/(H*L) * sum_h (sum_d q[b,l,h,:]) * (sum_d k[b,l,h,:]).
Then top-6 over l per batch, softmax, weighted sum of values rows -> [B,H,D].

Sharding: batch 16 -> 2 per core across 8 cores, no cross-core communication.

Per core (v4): q/k stream via HWDGE f32 in interleaved 0.5MB pieces (drain
starts ~1us earlier than SWDGE and the Sync sequencer is otherwise idle).
Per chunk: DVE reduce q/k, mul + head-reduce, and a single-column PE transpose
straight into a per-batch [1, 512] PSUM corr tile in natural l order -- no
SBUF rake DMA.  MAX8/FIND_INDEX8 read PSUM directly; FIND writes into the
32x32 index stage; one stream transpose makes the gather offset column;
per-batch fp16 indirect gathers (batch base via element_offset) overlap the
other batch's work; softmax exp+sum fused on ACT, weight scaling on ACT;
weighted sum is a single-pass fp16 matmul.  Batch 1 uses pipelined
partial-max8 (chunks 0-2 early, chunk 3 + merge late) so only FIND + gather +
matmul + store are exposed after the last byte.  Batch 0's whole tail hides
under batch 1's loads.  Emission order keeps the in-order DVE stream stall
free: batch-0 softmax staging is deferred until after batch-1's reduces.
"""

import numpy as np

import concourse.bass as bass
import concourse.mybir as mybir
import concourse.tile as tile
from concourse.masks import make_identity
from concourse.bass_utils import run_bass_kernel_spmd

B, L, H, D = 16, 512, 8, 64
HD = H * D                  # 512
NCORES = 8
BPC = B // NCORES           # 2 batches per core
ROWS = BPC * L              # 1024 rows of [HD] per core
P = 128
TPB = L // P                # 4 chunks per batch
KTOP = 6                    # k = int(log(512)) = 6
SCALE = 1.0 / (H * L)

_CACHE = {}


def _emit(tc, q, k, v, out):
    nc = tc.nc
    from contextlib import ExitStack

    with ExitStack() as ctx:
        main = ctx.enter_context(tc.tile_pool(name="main", bufs=1))
        small = ctx.enter_context(tc.tile_pool(name="small", bufs=1))
        psum = ctx.enter_context(tc.tile_pool(name="psum", bufs=1, space="PSUM"))

        q3 = q.rearrange("(t p) m -> t p m", p=P)
        k3 = k.rearrange("(t p) m -> t p m", p=P)

        # ---- all load DMAs up front (HWDGE f32, one FIFO ring).  q/k pieces
        # interleaved per batch so chunk compute starts early; batch 1's k
        # tail shrinks so the last reduce is short.
        qt = [main.tile([P, TPB, HD], mybir.dt.float32, tag=f"qt{b}", name=f"qt{b}") for b in range(BPC)]
        kt = [main.tile([P, TPB, HD], mybir.dt.float32, tag=f"kt{b}", name=f"kt{b}") for b in range(BPC)]
        ld = nc.sync.dma_start
        ld(out=qt[0][:, 0:1], in_=q3[0:1].rearrange("t p m -> p t m"))
        ld(out=kt[0][:, 0:1], in_=k3[0:1].rearrange("t p m -> p t m"))
        ld(out=qt[0][:, 1:2], in_=q3[1:2].rearrange("t p m -> p t m"))
        ld(out=kt[0][:, 1:2], in_=k3[1:2].rearrange("t p m -> p t m"))
        ld(out=qt[0][:, 2:4], in_=q3[2:4].rearrange("t p m -> p t m"))
        ld(out=kt[0][:, 2:3], in_=k3[2:3].rearrange("t p m -> p t m"))
        ld(out=kt[0][:, 3:4], in_=k3[3:4].rearrange("t p m -> p t m"))
        ld(out=qt[1][:, 0:2], in_=q3[4:6].rearrange("t p m -> p t m"))
        ld(out=kt[1][:, 0:2], in_=k3[4:6].rearrange("t p m -> p t m"))
        ld(out=qt[1][:, 2:4], in_=q3[6:8].rearrange("t p m -> p t m"))
        ld(out=kt[1][:, 2:3], in_=k3[6:7].rearrange("t p m -> p t m"))
        ld(out=kt[1][:, 3, 0 : HD // 2], in_=k3[7, :, 0 : HD // 2])
        ld(out=kt[1][:, 3, HD // 2 : HD], in_=k3[7, :, HD // 2 : HD])

        # identity + stage init after the loads so DMA issue starts first
        ident = small.tile([P, P], mybir.dt.float32)
        make_identity(nc, ident[:])

        psum_corr = [
            psum.tile([1, L], mybir.dt.float32, tag=f"pcorr{b}", name=f"pcorr{b}")
            for b in range(BPC)
        ]
        psum_out = [
            psum.tile([1, HD], mybir.dt.float32, tag=f"pout{b}", name=f"pout{b}")
            for b in range(BPC)
        ]
        junk = [
            small.tile([P, H], mybir.dt.float32, tag=f"junk{i}", name=f"junk{i}")
            for i in range(2)
        ]

        sq = [small.tile([P, TPB, H], mybir.dt.float32, tag=f"sq{b}", name=f"sq{b}") for b in range(BPC)]
        sk = [small.tile([P, TPB, H], mybir.dt.float32, tag=f"sk{b}", name=f"sk{b}") for b in range(BPC)]
        corr = [small.tile([P, TPB], mybir.dt.float32, tag=f"corr{b}", name=f"corr{b}") for b in range(BPC)]
        maxv = [small.tile([1, 8], mybir.dt.float32, tag=f"maxv{b}", name=f"maxv{b}") for b in range(BPC)]
        istage = [small.tile([32, 32], mybir.dt.uint32, tag=f"ist{b}", name=f"ist{b}") for b in range(BPC)]
        istageT = [small.tile([32, 32], mybir.dt.uint32, tag=f"istT{b}", name=f"istT{b}") for b in range(BPC)]
        wstage = [small.tile([32, 32], mybir.dt.float32, tag=f"wst{b}", name=f"wst{b}") for b in range(BPC)]
        wstageT = [small.tile([32, 32], mybir.dt.float32, tag=f"wstT{b}", name=f"wstT{b}") for b in range(BPC)]
        ssum = [small.tile([1, 1], mybir.dt.float32, tag=f"s{b}", name=f"s{b}") for b in range(BPC)]
        rsum = [small.tile([1, 1], mybir.dt.float32, tag=f"rs{b}", name=f"rs{b}") for b in range(BPC)]
        wcol16 = [
            small.tile([KTOP if b == 0 else 16, 1], mybir.dt.float16, tag=f"wc{b}", name=f"wc{b}")
            for b in range(BPC)
        ]
        gath16 = [
            small.tile([KTOP if b == 0 else 16, HD], mybir.dt.float16, tag=f"g16{b}", name=f"g16{b}")
            for b in range(BPC)
        ]
        istTb = small.tile([32, 32], mybir.dt.uint32, tag="istTb", name="istTb")
        istb = small.tile([32, 32], mybir.dt.uint32, tag="istb", name="istb")
        cand = small.tile([1, 16], mybir.dt.float32, tag="cand", name="cand")
        maxm = small.tile([1, 8], mybir.dt.float32, tag="maxm", name="maxm")
        e16 = small.tile([1, 16], mybir.dt.float32, tag="e16", name="e16")
        mask16 = small.tile([1, 16], mybir.dt.float32, tag="mask16", name="mask16")
        we16 = small.tile([1, 16], mybir.dt.float32, tag="we16", name="we16")
        outt = [small.tile([1, HD], mybir.dt.float32, tag=f"o{b}", name=f"o{b}") for b in range(BPC)]

        for b in range(BPC):
            nc.gpsimd.memset(istage[b][:], 0)
            nc.gpsimd.memset(wstage[b][:], 0.0)
        nc.gpsimd.memset(istb[:], 0)
        # dummy exp to pull ACT_TABLE_LOAD into the idle front window
        nc.scalar.activation(
            out=wstage[0][0:1, 0:1],
            in_=wstage[0][0:1, 0:1],
            func=mybir.ActivationFunctionType.Exp,
            scale=1.0,
        )

        def reduce_q(b, t):
            nc.vector.reduce_sum(
                out=sq[b][:, t, :],
                in_=qt[b][:, t].rearrange("p (h d) -> p h d", d=D),
                axis=mybir.AxisListType.X,
            )

        def reduce_k(b, t, half=None):
            if half is None:
                lo, hi = 0, HD
            else:
                lo, hi = half * (HD // 2), (half + 1) * (HD // 2)
            nc.vector.reduce_sum(
                out=sk[b][:, t, lo // D : hi // D],
                in_=kt[b][:, t, lo:hi].rearrange("p (h d) -> p h d", d=D),
                axis=mybir.AxisListType.X,
            )

        def corr_chunk(b, t):
            # corr[:, t] = sum_h sq[:, t, h] * sk[:, t, h]; the product runs
            # on the otherwise idle GpSimd engine, the 8-wide sum stays on
            # DVE, and a single-column PE transpose drops the result into
            # psum_corr[b][128t : 128t+128] in natural l order.
            j = junk[t % 2]
            nc.vector.tensor_mul(j[:], sq[b][:, t, :], sk[b][:, t, :])
            nc.vector.reduce_sum(
                out=corr[b][:, t : t + 1],
                in_=j[:],
                axis=mybir.AxisListType.X,
            )
            nc.tensor.transpose(
                out=psum_corr[b][:, P * t : P * (t + 1)],
                in_=corr[b][:, t : t + 1],
                identity=ident[:],
            )

        def find_and_gather(b):
            # maxv[b] holds the top-8; FIND writes indices into the stage,
            # one stream transpose makes the offset column, gather is fp16.
            nc.vector.max_index(
                out=istage[b][0:1, 0:8],
                in_max=maxv[b][:],
                in_values=psum_corr[b][:],
            )
            nc.scalar.activation(
                out=wstage[b][0:1, 0:KTOP],
                in_=maxv[b][:, 0:KTOP],
                func=mybir.ActivationFunctionType.Exp,
                scale=SCALE,
                accum_out=ssum[b][:],
            )
            nc.vector.transpose(out=istageT[b][:], in_=istage[b][:])
            nc.gpsimd.indirect_dma_start(
                out=gath16[b][:],
                out_offset=None,
                in_=v,
                in_offset=bass.IndirectOffsetOnAxis(
                    ap=istageT[b][0:KTOP, 0:1], axis=0
                ),
                element_offset=b * L * HD,
            )

        def weights(b):
            # softmax normalize on ACT (scale=1/sum via AP), transpose to a
            # column, cast fp16 for the matmul lhsT.
            nc.vector.reciprocal(out=rsum[b][:], in_=ssum[b][:])
            nc.scalar.mul(
                wstage[b][0:1, 0:KTOP], wstage[b][0:1, 0:KTOP], rsum[b][:, 0:1]
            )
            nc.vector.transpose(out=wstageT[b][:], in_=wstage[b][:])
            nc.scalar.copy(wcol16[b][:], wstageT[b][0:KTOP, 0:1])

        def matmul_out(b):
            nc.tensor.matmul(
                out=psum_out[b][:],
                lhsT=wcol16[b][:],
                rhs=gath16[b][:],
                start=True,
                stop=True,
            )

        def store(b):
            if b == 0:
                nc.scalar.copy(outt[b][:], psum_out[b][:])
            else:
                nc.vector.tensor_copy(outt[b][:], psum_out[b][:])
            nc.sync.dma_start(out=out[b : b + 1, :], in_=outt[b][:])

        # ---- batch 0 compute (chunk-pipelined, per-chunk pieces for t0/t1)
        reduce_q(0, 0)
        reduce_k(0, 0)
        corr_chunk(0, 0)
        reduce_q(0, 1)
        reduce_k(0, 1)
        corr_chunk(0, 1)
        for t in range(2, 4):
            reduce_q(0, t)
        for t in range(2, 4):
            reduce_k(0, t)
            corr_chunk(0, t)
        nc.vector.max(out=maxv[0][:], in_=psum_corr[0][:])
        find_and_gather(0)

        # ---- batch 1 reduces (batch 0's gather runs under these)
        for t in range(2):
            reduce_q(1, t)
        for t in range(2):
            reduce_k(1, t)
            corr_chunk(1, t)
        for t in range(2, 4):
            reduce_q(1, t)
        reduce_k(1, 2)
        corr_chunk(1, 2)
        reduce_k(1, 3, half=0)
        reduce_k(1, 3, half=1)

        # top-8 of chunks 0-2 + their gather, off the chunk-3 critical path
        nc.vector.max(out=cand[:, 0:8], in_=psum_corr[1][:, 0 : 3 * P])
        nc.vector.max_index(
            out=istage[1][0:1, 0:8],
            in_max=cand[:, 0:8],
            in_values=psum_corr[1][:, 0 : 3 * P],
        )
        nc.vector.transpose(out=istageT[1][:], in_=istage[1][:])
        nc.gpsimd.indirect_dma_start(
            out=gath16[1][0:8],
            out_offset=None,
            in_=v,
            in_offset=bass.IndirectOffsetOnAxis(ap=istageT[1][0:8, 0:1], axis=0),
            element_offset=L * HD,
        )

        corr_chunk(1, 3)
        # top-8 of chunk 3 only (128 values): short late scan
        nc.vector.max(out=cand[:, 8:16], in_=psum_corr[1][:, 3 * P : L])
        nc.vector.max_index(
            out=istb[0:1, 0:8],
            in_max=cand[:, 8:16],
            in_values=psum_corr[1][:, 3 * P : L],
        )
        nc.vector.transpose(out=istTb[:], in_=istb[:])
        nc.gpsimd.indirect_dma_start(
            out=gath16[1][8:16],
            out_offset=None,
            in_=v,
            in_offset=bass.IndirectOffsetOnAxis(ap=istTb[0:8, 0:1], axis=0),
            element_offset=(L + 3 * P) * HD,
        )

        # merged softmax over the 16 candidates (runs under the gather):
        # the 6th largest of cand is the threshold; mask kills the rest.
        nc.vector.max(out=maxm[:], in_=cand[:])
        nc.vector.tensor_scalar(
            mask16[:], cand[:], maxm[0:1, 5:6], None, op0=mybir.AluOpType.is_ge
        )
        nc.scalar.activation(
            out=e16[:],
            in_=cand[:],
            func=mybir.ActivationFunctionType.Exp,
            scale=SCALE,
        )
        nc.vector.tensor_mul(we16[:], e16[:], mask16[:])
        nc.vector.reduce_sum(out=ssum[1][:], in_=we16[:], axis=mybir.AxisListType.X)
        nc.vector.reciprocal(out=rsum[1][:], in_=ssum[1][:])
        nc.vector.tensor_scalar_mul(
            wstage[1][0:1, 0:16], we16[:], rsum[1][:, 0:1]
        )
        nc.vector.transpose(out=wstageT[1][:], in_=wstage[1][:])
        nc.vector.tensor_copy(wcol16[1][:], wstageT[1][0:16, 0:1])

        weights(0)
        matmul_out(0)
        store(0)
        matmul_out(1)
        store(1)


def _build_bass():
    import concourse.bacc as bacc

    nc = bacc.Bacc(trn_type="TRN2", target_bir_lowering=False, debug=False)
    q = nc.dram_tensor("q", [ROWS, HD], mybir.dt.float32, kind="ExternalInput").ap()
    k = nc.dram_tensor("k", [ROWS, HD], mybir.dt.float32, kind="ExternalInput").ap()
    v = nc.dram_tensor("v", [ROWS, HD], mybir.dt.float32, kind="ExternalInput").ap()
    out = nc.dram_tensor(
        "out", [BPC, HD], mybir.dt.float32, kind="ExternalOutput"
    ).ap()
    with tile.TileContext(nc) as tc:
        _emit(tc, q, k, v, out)
    nc.compile()
    return nc


def _get_nc():
    if "nc" not in _CACHE:
        _CACHE["nc"] = _build_bass()
    return _CACHE["nc"]


def run_sharded(queries, keys, values, trace=False, **kw):
    """Shard over 8 cores, run, gather. Returns (out [16,8,64], BassKernelResults)."""
    nc = _get_nc()
    q = np.ascontiguousarray(np.asarray(queries, dtype=np.float32))
    k = np.ascontiguousarray(np.asarray(keys, dtype=np.float32))
    v = np.ascontiguousarray(np.asarray(values, dtype=np.float32))
    in_maps = []
    for c in range(NCORES):
        sl = slice(c * BPC, (c + 1) * BPC)
        in_maps.append(
            {
                "q": q[sl].reshape(ROWS, HD),
                "k": k[sl].reshape(ROWS, HD),
                "v": v[sl].reshape(ROWS, HD),
            }
        )
    res = run_bass_kernel_spmd(nc, in_maps, list(range(NCORES)), trace=trace, **kw)
    out = np.empty((B, H, D), dtype=np.float32)
    for c in range(NCORES):
        out[c * BPC : (c + 1) * BPC] = res.results[c]["out"].reshape(BPC, H, D)
    return out, res


def kernel(queries, keys, values, B=None, **_ignored):
    out, _ = run_sharded(queries, keys, values, trace=False)
    return out
